# revision 18
# baseline (speedup 1.0000x reference)
"""GCN layer (GCNConv + per-channel PReLU) on 8 Trainium2 NeuronCores.

out = PReLU(D^-1/2 (A+I) D^-1/2 (x @ W) + bias)

v2 design (dst-sharded graph parallelism, gather-stream pipelined):
  * Host folds dinv[src] into x columns (xt), buckets edges per core into
    (dst-window, src-chunk) padded 128-edge blocks.  Self loops are NOT
    edges: each core gets its shard's x columns (xself) and the device
    recomputes the self term, added in the epilogue.
  * Device phase 1 (replicated): h2 = (x*dinv) @ W as bf16 row tables in
    DRAM, one tensor per src chunk; PSUM evacuated on the scalar engine.
  * Main pass: one continuous dma_gather stream (4 SWDGE queues) in global
    chunk-major order for chunks 0..2 (overlapping phase 1; per-group PSUM
    drained into an SBUF f32 accumulator), then stripe-major for chunks
    3..5 (PSUM accumulates across chunks; epilogue merges acc + psum +
    self term, applies dinv[dst]/bias/PReLU).
  * Aggregation per 128-edge block: one-hot S[e, d] built on the vector
    engine, S^T @ msgs on the tensor engine.
"""

import sys
from contextlib import ExitStack

import numpy as np

try:
    import concourse.bass as bass
except ImportError:  # pragma: no cover
    sys.path.insert(0, "/opt/trn_rl_repo")
    import concourse.bass as bass

import concourse.tile as tile
from concourse import bacc, mybir
from concourse.bass_utils import run_bass_kernel_spmd

BF16 = mybir.dt.bfloat16
F32 = mybir.dt.float32
I16 = mybir.dt.int16
NP_BF16 = mybir.dt.np(BF16)

P = 128
NCORES = 8
STRIPE_W = 8     # dst windows per stripe: psum tile [P, 8*128] f32 = 2 banks
CHUNK = 17408    # 1024-aligned, < 32767 (int16 gather indices)
PH1_G = 4        # phase-1 row blocks per PSUM bank
SG = 2 * PH1_G   # phase-1 super-group (one x load / h2 write)
SBUILD = "tt_bcast"  # S one-hot build variant (see _build)

MULT = mybir.AluOpType.mult
ADD = mybir.AluOpType.add
ISEQ = mybir.AluOpType.is_equal

last_results = None  # BassKernelResults of the most recent run (for profiling)


def _ceil(a, b):
    return -(-a // b)


def _plan(x, edge_index, W):
    """Host-side sharding/bucketing. Returns (meta, shared, per_core)."""
    N, IN = x.shape
    HID = W.shape[1]
    assert HID == P and IN % P == 0 and N % NCORES == 0
    SHARD = N // NCORES
    NWIN = _ceil(SHARD, P)
    NPAD = _ceil(N, P) * P
    NBLK = NPAD // P
    KH = IN // P
    NCHUNK = _ceil(NPAD, CHUNK)
    NSECA = NCHUNK // 2  # chunks 0..NSECA-1: chunk-major + SBUF accumulator

    src = np.asarray(edge_index[0]).astype(np.int64)
    dst = np.asarray(edge_index[1]).astype(np.int64)
    deg = np.bincount(dst, minlength=N).astype(np.float64) + 1.0
    dinv = (1.0 / np.sqrt(deg)).astype(np.float32)

    core = dst // SHARD
    dstl = dst - core * SHARD
    w_ = dstl // P
    dloc = (dstl % P).astype(np.float32)
    ch_ = src // CHUNK
    sloc = (src - ch_ * CHUNK).astype(np.int16)
    NB = NWIN * NCHUNK
    bucket = w_ * NCHUNK + ch_

    counts = np.zeros((NCORES, NB), np.int64)
    for c in range(NCORES):
        counts[c] = np.bincount(bucket[core == c], minlength=NB)
    Tb = np.maximum(_ceil(counts.max(axis=0), P), 1)  # shared schedule

    stripes = [list(range(i, min(i + STRIPE_W, NWIN)))
               for i in range(0, NWIN, STRIPE_W)]

    # group emission order: chunks 0..NSECA-1 chunk-major (overlap phase 1,
    # drained to an SBUF accumulator), the rest stripe-major (psum
    # accumulates across those chunks until the epilogue)
    order = [(si, c) for c in range(NSECA) for si in range(len(stripes))]
    order += [(si, c) for si in range(len(stripes))
              for c in range(NSECA, NCHUNK)]

    base = np.full(NB, -1, np.int64)
    tot = 0
    groups = []  # dicts: si, ch, blk_base, nblk, mm=[(w_local, blk, start, stop)]
    for (si, c_) in order:
        st = stripes[si]
        mm = []
        blk_base = tot // P
        nb = 0
        for wl, w in enumerate(st):
            tb = int(Tb[w * NCHUNK + c_])
            base[w * NCHUNK + c_] = tot
            for t in range(tb):
                if c_ < NSECA:
                    fl_start, fl_stop = t == 0, t == tb - 1
                else:
                    fl_start = c_ == NSECA and t == 0
                    fl_stop = c_ == NCHUNK - 1 and t == tb - 1
                mm.append((wl, nb + t, fl_start, fl_stop))
            nb += tb
            tot += tb * P
        groups.append(dict(si=si, ch=c_, blk_base=blk_base, nblk=nb, mm=mm))
    TOTSLOT = tot
    TOTBLK = tot // P
    NBMAX = max(g["nblk"] for g in groups)

    # per-core edge arrays
    idx_list, dval_list, ddst_list, xself_list = [], [], [], []
    dinv_pad = np.ones(NPAD, np.float32)
    dinv_pad[:N] = dinv
    xtf = np.zeros((IN, NPAD), np.float32)
    xtf[:, :N] = np.asarray(x, np.float32).T * dinv[None, :]
    xt = np.ascontiguousarray(xtf.astype(NP_BF16))

    for c in range(NCORES):
        m = core == c
        b_c = bucket[m]
        s_c = sloc[m]
        d_c = dloc[m]
        o = np.argsort(b_c, kind="stable")
        b_s = b_c[o]
        cnt = counts[c]
        cum = np.concatenate([[0], np.cumsum(cnt)[:-1]])
        rank = np.arange(b_s.size) - cum[b_s]
        slot = base[b_s] + rank
        sidx = np.zeros(TOTSLOT, np.int16)
        dval = np.full(TOTSLOT, -1.0, np.float32)
        sidx[slot] = s_c[o]
        dval[slot] = d_c[o]
        idx_list.append(np.ascontiguousarray(
            np.tile(sidx.reshape(-1, 16).T, (8, 1))))          # [128, TOTSLOT//16] i16
        dval_list.append(np.ascontiguousarray(
            dval.reshape(-1, P).T.astype(NP_BF16)))            # [128, TOTBLK] bf16
        dvp = np.ones(NWIN * P, np.float32)
        dvp[:SHARD] = dinv[c * SHARD:(c + 1) * SHARD]
        ddst_list.append(np.ascontiguousarray(dvp.reshape(NWIN, P).T))  # [128, NWIN]
        xself_list.append(np.ascontiguousarray(
            xt[:, c * SHARD:c * SHARD + NWIN * P]))            # [IN, NWIN*P] bf16

    meta = dict(N=N, IN=IN, HID=HID, SHARD=SHARD, NWIN=NWIN, NPAD=NPAD,
                NBLK=NBLK, KH=KH, NCHUNK=NCHUNK, NSECA=NSECA,
                TOTSLOT=TOTSLOT, TOTBLK=TOTBLK,
                NBMAX=NBMAX, stripes=stripes, groups=groups)
    shared = dict(xt=xt)
    per_core = dict(idx16=idx_list, dval=dval_list, dinv_dst=ddst_list,
                    xself=xself_list)
    return meta, shared, per_core


def _bcast_inner(ap, n_outer, n_inner):
    return ap.to_broadcast([P, n_outer, n_inner])


def _bcast_outer(ap, n_outer):
    """[P, n_outer, inner] view of a [P, inner] AP, broadcast on outer dim."""
    return bass.AP(tensor=ap.tensor, offset=ap.offset,
                   ap=[list(ap.ap[0]), [0, n_outer], list(ap.ap[1])])


def _build(meta):
    """Build the SPMD bass program (shared by all 8 cores)."""
    IN, HID = meta["IN"], meta["HID"]
    NWIN, NPAD, NBLK, KH = meta["NWIN"], meta["NPAD"], meta["NBLK"], meta["KH"]
    NCHUNK, NSECA = meta["NCHUNK"], meta["NSECA"]
    TOTSLOT, TOTBLK, NBMAX = meta["TOTSLOT"], meta["TOTBLK"], meta["NBMAX"]
    stripes, groups = meta["stripes"], meta["groups"]
    CBLK = CHUNK // P          # 136 table blocks per chunk
    assert CBLK % SG == 0       # chunk boundary aligns with supergroups

    nc = bacc.Bacc("TRN2", target_bir_lowering=False, debug=False,
                   num_devices=NCORES, num_swdge_queues=4)

    xt = nc.dram_tensor("xt", [IN, NPAD], BF16, kind="ExternalInput").ap()
    xself = nc.dram_tensor("xself", [IN, NWIN * P], BF16, kind="ExternalInput").ap()
    w2 = nc.dram_tensor("w2", [IN, HID], BF16, kind="ExternalInput").ap()
    bias2 = nc.dram_tensor("bias2", [1, HID], F32, kind="ExternalInput").ap()
    alpha2 = nc.dram_tensor("alpha2", [1, HID], F32, kind="ExternalInput").ap()
    iota2 = nc.dram_tensor("iota2", [1, P], BF16, kind="ExternalInput").ap()
    ddst = nc.dram_tensor("ddst", [P, NWIN], F32, kind="ExternalInput").ap()
    idx16 = nc.dram_tensor("idx16", [P, TOTSLOT // 16], I16, kind="ExternalInput").ap()
    dvalb = nc.dram_tensor("dvalb", [P, TOTBLK], BF16, kind="ExternalInput").ap()

    # gather tables: one DRAM tensor per chunk (gather source must be offset 0)
    h2c = [nc.dram_tensor(f"h2_{c}", [CHUNK, HID], BF16) for c in range(NCHUNK)]
    h2self = nc.dram_tensor("h2self", [NWIN * P, HID], BF16)
    outp = nc.dram_tensor("outp", [NWIN * P, HID], F32, kind="ExternalOutput").ap()

    h2cr = [h.ap().rearrange("(b p) c -> p b c", p=P) for h in h2c]
    h2selfr = h2self.ap().rearrange("(b p) c -> p b c", p=P)
    outr = outp.rearrange("(b p) c -> p b c", p=P)

    with ExitStack() as ctx:
        tc = ctx.enter_context(tile.TileContext(nc))
        consts = ctx.enter_context(tc.tile_pool(name="consts", bufs=1))

        # --- resident constants (sync ring) -------------------------------
        w_sb = consts.tile([P, KH, HID], BF16)
        for kk in range(KH):
            nc.sync.dma_start(out=w_sb[:, kk, :], in_=w2[kk * P:(kk + 1) * P, :])
        iota_sb = consts.tile([P, P], BF16)
        nc.sync.dma_start(out=iota_sb[:], in_=bass.AP(
            tensor=iota2.tensor, offset=iota2.offset, ap=[[0, P], [1, P]]))
        bias_sb = consts.tile([P, HID], F32)
        nc.sync.dma_start(out=bias_sb[:], in_=bass.AP(
            tensor=bias2.tensor, offset=bias2.offset, ap=[[0, P], [1, HID]]))
        alpha_sb = consts.tile([P, HID], F32)
        nc.sync.dma_start(out=alpha_sb[:], in_=bass.AP(
            tensor=alpha2.tensor, offset=alpha2.offset, ap=[[0, P], [1, HID]]))
        ddst_sb = consts.tile([P, NWIN], F32)
        nc.sync.dma_start(out=ddst_sb[:], in_=ddst[:, :])
        dvalb_sb = consts.tile([P, TOTBLK], BF16)
        nc.sync.dma_start(out=dvalb_sb[:], in_=dvalb[:, :])

        # --- main-pass pools (allocated first: fresh SBUF, no phase-1 WAR) -
        acc = consts.tile([P, NWIN, P], F32, name="acc")
        mmsg = ctx.enter_context(tc.tile_pool(name="mmsg", bufs=7))
        msel = ctx.enter_context(tc.tile_pool(name="msel", bufs=6))
        midx = ctx.enter_context(tc.tile_pool(name="midx", bufs=10))
        mep = ctx.enter_context(tc.tile_pool(name="mep", bufs=1))
        mpsum = ctx.enter_context(tc.tile_pool(name="mpsum", bufs=3, space="PSUM"))

        # --- phase 1: h2 = xt @ W (dinv[src] pre-folded on host) -----------
        # all DMA + psum evacuation on the scalar (ACT) engine; the sync ring
        # is reserved for the gather stream's idx loads
        with tc.tile_pool(name="p1x", bufs=2) as p1x, \
             tc.tile_pool(name="p1h", bufs=2) as p1h, \
             tc.tile_pool(name="p1ps", bufs=2, space="PSUM") as p1ps:

            def mm_rows(src_ap, ncols, n_blocks, j, dst_rearr, cblk_tab):
                g = min(SG, n_blocks - j * SG)
                col0 = j * SG * P
                xp = p1x.tile([P, KH, SG * P], BF16, tag="xp")
                xt3 = bass.AP(
                    tensor=src_ap.tensor, offset=src_ap.offset + col0,
                    ap=[[ncols, P], [P * ncols, KH], [1, g * P]])
                nc.scalar.dma_start(out=xp[:, 0:KH, 0:g * P], in_=xt3)
                h2t = p1h.tile([P, SG, P], BF16, tag="h2t")
                for half in range(_ceil(g, PH1_G)):
                    gh = min(PH1_G, g - half * PH1_G)
                    ps = p1ps.tile([P, PH1_G * P], F32, tag="ps", name="ps")
                    for k in range(gh):
                        kb = half * PH1_G + k
                        for kk in range(KH):
                            nc.tensor.matmul(out=ps[:, k * P:(k + 1) * P],
                                             lhsT=xp[:, kk, kb * P:(kb + 1) * P],
                                             rhs=w_sb[:, kk, :],
                                             start=(kk == 0), stop=(kk == KH - 1))
                    # evacuate on the scalar engine (ACT), bf16 out
                    nc.scalar.activation(
                        out=h2t[:, half * PH1_G:half * PH1_G + gh, :],
                        in_=ps[:, 0:gh * P].rearrange("p (g q) -> p g q", q=P),
                        func=mybir.ActivationFunctionType.Copy)
                blk0 = j * SG
                ci, cb = blk0 // cblk_tab, blk0 % cblk_tab
                nc.scalar.dma_start(out=dst_rearr(ci)[:, cb:cb + g, :],
                                    in_=h2t[:, 0:g, :])

            for j in range(_ceil(NBLK, SG)):
                mm_rows(xt, NPAD, NBLK, j, lambda ci: h2cr[ci], CBLK)
            # phase 1b: self-term table from this core's shard columns
            for j in range(_ceil(NWIN, SG)):
                mm_rows(xself, NWIN * P, NWIN, j, lambda ci: h2selfr, NWIN + SG)

        # --- main pass ----------------------------------------------------
        # 1) gather stream: sidx loads on the sync ring + dma_gather on the
        #    4 SWDGE queues, in global group order (chunk-major first)
        g_tiles = {}
        for gi, grp in enumerate(groups):
            nb = grp["nblk"]
            b0 = grp["blk_base"]
            sidx = midx.tile([P, NBMAX * 8], I16, tag="sidx")
            nc.sync.dma_start(out=sidx[:, 0:nb * 8],
                              in_=idx16[:, b0 * 8:(b0 + nb) * 8])
            mt = mmsg.tile([P, NBMAX, P], BF16, tag="mt")
            nc.gpsimd.dma_gather(
                mt[:, 0:nb, :],
                h2c[grp["ch"]].ap()[:, :],
                sidx[:, 0:nb * 8],
                nb * P, nb * P, P,
                single_packet=False,
                queue_num=gi % 4,
            )
            g_tiles[gi] = mt

        # 2) S-build + matmuls + drains/epilogues, same group order.
        # PSUM allows only ONE open accumulation group per bank at a time, so
        # every window's start..stop matmul run must complete before the next
        # window in the same psum tile starts.  Section A groups are
        # per-window sequential already; section B (chunks NSECA..) must
        # therefore consume its per-chunk groups WINDOW-major.
        ps_tiles = {}
        b_stash = {}  # si -> list of (grp, st_, mt)
        for gi, grp in enumerate(groups):
            si, c_, nb, b0 = grp["si"], grp["ch"], grp["nblk"], grp["blk_base"]
            st = stripes[si]
            NW = len(st)
            mt = g_tiles[gi]

            # one-hot S [slot, dst-local] per block (vector engine)
            st_ = msel.tile([P, NBMAX, P], BF16, tag="st")
            nc.vector.tensor_tensor(
                out=st_[:, 0:nb, :],
                in0=_bcast_inner(dvalb_sb[:, b0:b0 + nb], nb, P),
                in1=_bcast_outer(iota_sb[:], nb),
                op=ISEQ)

            if c_ < NSECA:
                ps = mpsum.tile([P, STRIPE_W, P], F32, tag="ps", name="mps")
                for (wl, b, fl_start, fl_stop) in grp["mm"]:
                    nc.tensor.matmul(out=ps[:, wl, :],
                                     lhsT=st_[:, b, :], rhs=mt[:, b, :],
                                     start=fl_start, stop=fl_stop,
                                     skip_group_check=True)
                # drain into the SBUF accumulator
                w0 = st[0]
                acc_sl = acc[:, w0:w0 + NW, :]
                if c_ == 0:
                    nc.vector.tensor_scalar(
                        out=acc_sl, in0=ps[:, 0:NW, :], scalar1=0.0,
                        scalar2=None, op0=ADD)
                else:
                    nc.vector.tensor_tensor(
                        out=acc_sl, in0=ps[:, 0:NW, :], in1=acc_sl, op=ADD)
                continue

            b_stash.setdefault(si, []).append((grp, st_, mt))
            if c_ == NCHUNK - 1:
                ps = mpsum.tile([P, STRIPE_W, P], F32, tag="ps", name="mps")
                for wl in range(NW):
                    for (g2, st2, mt2) in b_stash[si]:
                        for (wl2, b, fl_start, fl_stop) in g2["mm"]:
                            if wl2 != wl:
                                continue
                            nc.tensor.matmul(out=ps[:, wl, :],
                                             lhsT=st2[:, b, :], rhs=mt2[:, b, :],
                                             start=fl_start, stop=fl_stop,
                                             skip_group_check=True)
                del b_stash[si]
                # epilogue for this stripe
                w0 = st[0]
                selft = mep.tile([P, STRIPE_W, P], BF16, tag="selft")
                nc.scalar.dma_start(out=selft[:, 0:NW, :],
                                    in_=h2selfr[:, w0:w0 + NW, :])
                t1 = mep.tile([P, STRIPE_W, P], F32, tag="t1")
                if NSECA > 0:
                    nc.vector.tensor_tensor(out=t1[:, 0:NW, :], in0=ps[:, 0:NW, :],
                                            in1=acc[:, w0:w0 + NW, :], op=ADD)
                    nc.vector.tensor_tensor(out=t1[:, 0:NW, :], in0=t1[:, 0:NW, :],
                                            in1=selft[:, 0:NW, :], op=ADD)
                else:
                    nc.vector.tensor_tensor(out=t1[:, 0:NW, :], in0=ps[:, 0:NW, :],
                                            in1=selft[:, 0:NW, :], op=ADD)
                for wl, w in enumerate(st):
                    nc.vector.scalar_tensor_tensor(
                        out=t1[:, wl, :], in0=t1[:, wl, :],
                        scalar=ddst_sb[:, w:w + 1], in1=bias_sb[:],
                        op0=MULT, op1=ADD)
                ot = mep.tile([P, STRIPE_W, P], F32, tag="ot")
                if meta.get("alpha_01", False):
                    mn = mep.tile([P, STRIPE_W, P], F32, tag="mn")
                    nc.vector.tensor_tensor(
                        out=mn[:, 0:NW, :], in0=t1[:, 0:NW, :],
                        in1=_bcast_outer(alpha_sb[:], NW), op=MULT)
                    nc.vector.tensor_tensor(out=ot[:, 0:NW, :], in0=t1[:, 0:NW, :],
                                            in1=mn[:, 0:NW, :],
                                            op=mybir.AluOpType.max)
                else:
                    mx = mep.tile([P, STRIPE_W, P], F32, tag="mx")
                    nc.vector.tensor_scalar_max(mx[:, 0:NW, :], t1[:, 0:NW, :], 0.0)
                    mn = mep.tile([P, STRIPE_W, P], F32, tag="mn")
                    nc.vector.tensor_scalar_min(mn[:, 0:NW, :], t1[:, 0:NW, :], 0.0)
                    nc.vector.tensor_tensor(out=mn[:, 0:NW, :], in0=mn[:, 0:NW, :],
                                            in1=_bcast_outer(alpha_sb[:], NW),
                                            op=MULT)
                    nc.vector.tensor_tensor(out=ot[:, 0:NW, :], in0=mx[:, 0:NW, :],
                                            in1=mn[:, 0:NW, :], op=ADD)
                nc.sync.dma_start(out=outr[:, w0:w0 + NW, :], in_=ot[:, 0:NW, :])

    _hoist_reg_moves(nc)
    return nc


def _hoist_reg_moves(nc):
    """Tile defers constant reg-writes (to_reg) and does not track the
    register dependency of custom instructions like InstDMAGatherAnt, so the
    defining InstRegisterMove can land after its use. Hoist each such move to
    just before the first use of its register within the block."""
    for bb in nc.m.functions[0].blocks:
        insts = bb.instructions
        use_pos = {}
        movs = []
        for i, ins in enumerate(insts):
            for a in ins.ins:
                if isinstance(a, mybir.RegisterAccess):
                    use_pos.setdefault(a.regref, i)
            if isinstance(ins, mybir.InstRegisterMove):
                outs = list(ins.outs)
                if outs and isinstance(outs[0], mybir.RegisterAccess):
                    movs.append((i, outs[0].regref, ins))
        for i, regref, ins in sorted(movs, reverse=True):
            first_use = use_pos.get(regref)
            if first_use is not None and first_use < i:
                del insts[i]
                insts.insert(first_use, ins)


def kernel(x, edge_index, W, bias, alpha):
    global last_results
    x = np.asarray(x)
    edge_index = np.asarray(edge_index)
    W = np.asarray(W)
    bias = np.asarray(bias, dtype=np.float32)
    alpha = np.asarray(alpha, dtype=np.float32)

    meta, shared, per_core = _plan(x, edge_index, W)
    meta["alpha_01"] = bool(np.all((alpha >= 0.0) & (alpha <= 1.0)))
    nc = _build(meta)
    if not nc.is_finalized():
        nc.finalize()

    w2 = np.ascontiguousarray(np.asarray(W, np.float32).astype(NP_BF16))
    bias2 = bias.reshape(1, -1)
    alpha2 = alpha.reshape(1, -1)
    iota2 = np.arange(P, dtype=np.float32).astype(NP_BF16).reshape(1, P)

    in_maps = []
    for c in range(NCORES):
        in_maps.append(dict(
            xt=shared["xt"], xself=per_core["xself"][c], w2=w2, bias2=bias2,
            alpha2=alpha2, iota2=iota2, ddst=per_core["dinv_dst"][c],
            idx16=per_core["idx16"][c], dvalb=per_core["dval"][c],
        ))

    res = run_bass_kernel_spmd(nc, in_maps, core_ids=list(range(NCORES)))
    last_results = res
    SHARD = meta["SHARD"]
    out = np.concatenate([res.results[c]["outp"][:SHARD] for c in range(NCORES)],
                         axis=0)
    return out.astype(np.float32)
